# revision 41
# baseline (speedup 1.0000x reference)
"""EMRouting2d Trainium2 kernel (8-core SPMD, data-parallel over batch).

Cost-model estimate ~438 us/core (prior baseline 553, stub 772).

Reference computation (per batch item b, handled by core b):
  con[g, c, o, p] = sum_i w[c*16+o, g*8+i] * x[g*8+i, p]        (grouped 1x1 conv)
  EM loop over ITERS=3: soft-assignment r over clusters c, reductions over
  groups g, final output mean[c*16+o, p] + bias.

Layout per core: SBUF partitions p = o*8 + g_lo (o in 16, g_lo in 8);
free dims (c in 3, gh in 4, N=256 pixels/chunk), 16 chunks, big tensors fp16
(2x DVE tensor_tensor throughput). con via f32r PE matmuls (1 cyc/row).

Per round (2 softmax rounds; final mean from round-1 reductions):
  d   = con - mean_bc          TT (DVE; rnd1 c0 on Pool, c1/c2 DVE)
  sq  = Square(d) in-place     (Act) -- round 0: PE-reduces sq for var0,
                               eliminating the con^2 pass entirely
  u   = sq * hv_bc             TT (DVE), hv = 1/(2 var) via Exp(-ln var + ln .5)
  E   = Exp(-u + SHIFT)        (Act, in-place, per-partition bias)
  n   = E * eu_bc              TT (DVE, in-place), eu = rk*var^-.5*EU_SCALE
                               via Exp(-.5 ln var + ln eu_k) -- no separate mult
  den = (n0+n1)+n2 (f32 Pool); q = reciprocal_approx_fast (DVE);
        den+recip split into gh-halves so recip starts on half the tile
  m   = n * q_bc               (Pool c01 / DVE c2) -> r_ik
  mc  = m * con                TT (DVE)
  mtt = m * sq                 (rnd0 only; Pool c0,c1 / DVE c2)
  PE reductions (fp16 selection matmuls, c-order 2,0,1):
    rk = sum_g m, S1 = sum_g mc, T0 = sum_g mtt
Round-1 stats use the centered-variance fixup (no second-moment pass):
    var1 = T0/rk1 - (mean1-mean0)^2 + EPS
Small [96,N] eps/bias/scale stats ride the Act engine (Identity/Copy with
per-partition bias APs); PSUM split into 6 short-lived 1-bank tags so the
stats pipeline is multiple chunks deep; x DMA merged; all broadcast DMAs on
the SP HWDGE queue.  Software pipeline: WINDOW=5 chunks, STAGGER=6 steps.
Note: the Pool engine only supports TensorTensor/Memset/ISA on real TRN2
hardware (no TensorScalarPtr), and TensorScalarPtr APs must be <=3 dims.
"""

import os
import sys
import numpy as np

for _p in ("/opt/trn_rl_repo", "/opt/pypackages"):
    if _p not in sys.path:
        sys.path.insert(0, _p)

import concourse.bass as bass
import concourse.bacc as bacc
import concourse.tile as tile
from concourse import mybir
from concourse.bass_utils import run_bass_kernel_spmd

F32 = mybir.dt.float32
F32R = mybir.dt.float32r
F16 = mybir.dt.float16

EPS = 1e-7
G, I, C, O, ITERS = 32, 8, 3, 16, 3
B, H, W = 8, 64, 64
HW = H * W
GI = G * I
CO = C * O

N = 256
NCHUNK = HW // N

SHIFT = 8.0
EU_SCALE = 1.0 / 256.0
LOG_HALF = float(np.log(0.5))
AUXW = 4 * C * 128 + 2 * 96


def build_program():
    nc = bacc.Bacc("TRN2", target_bir_lowering=False, debug=False)

    x_d = nc.dram_tensor("x", [GI, HW], F32R, kind="ExternalInput").ap()
    aux_d = nc.dram_tensor("aux", [128, AUXW], F32R,
                           kind="ExternalInput").ap()
    auxf_d = nc.dram_tensor("auxf", [128, 5], F32, kind="ExternalInput").ap()
    selred_d = nc.dram_tensor("selred", [128, 3 * 96], F16,
                              kind="ExternalInput").ap()
    out_d = nc.dram_tensor("out", [CO, HW], F32, kind="ExternalOutput").ap()

    with tile.TileContext(nc) as tc:
        _body(tc, x_d, aux_d, auxf_d, selred_d, out_d)
    nc.compile()
    return nc


def _body(tc, x_d, aux_d, auxf_d, selred_d, out_d):
    nc = tc.nc
    mm = mybir.AluOpType.mult
    ad = mybir.AluOpType.add
    sb = mybir.AluOpType.subtract
    EXP = mybir.ActivationFunctionType.Exp
    LN = mybir.ActivationFunctionType.Ln
    SQ = mybir.ActivationFunctionType.Square

    WIN = int(os.environ.get("EMWIN", "5"))
    HIPRI = int(os.environ.get("EMHIPRI", "0")) or None
    if os.environ.get("EMHIPRI", "0") == "0":
        HIPRI = 0
    from contextlib import ExitStack
    ctx = ExitStack()
    wpool = ctx.enter_context(tc.tile_pool(name="wpool", bufs=1))
    xin = ctx.enter_context(tc.tile_pool(name="xin", bufs=WIN + 2))
    conp = ctx.enter_context(tc.tile_pool(name="conp", bufs=2))
    big = ctx.enter_context(tc.tile_pool(name="big", bufs=2))
    bcp = ctx.enter_context(tc.tile_pool(name="bcp", bufs=3))
    small = ctx.enter_context(tc.tile_pool(name="small", bufs=3))
    pcm = ctx.enter_context(tc.tile_pool(name="pcm", bufs=2, space="PSUM"))
    pstat = ctx.enter_context(tc.tile_pool(name="pstat", bufs=1, space="PSUM"))

    # persistent weights
    auxt = wpool.tile([128, AUXW], F32R)
    nc.sync.dma_start(auxt[:], aux_d)
    auxf = wpool.tile([128, 5], F32)
    nc.sync.dma_start(auxf[:], auxf_d)
    wcon = auxt[:, 0:4 * C * 128].rearrange("k (g c m) -> k g c m", g=4, c=C)
    wred = auxt[:, 4 * C * 128:4 * C * 128 + 192].rearrange(
        "k (h m) -> k h m", h=2)
    shiftc = auxf[:, 0:1]
    biasc = auxf[:96, 1:2]
    lneukc = auxf[:96, 2:3]
    loghalfc = auxf[:96, 3:4]
    epsc = auxf[:96, 4:5]
    selred = wpool.tile([128, 3, 96], F16)
    nc.sync.dma_start(selred[:], selred_d.rearrange("k (c m) -> k c m", c=3))
    nc.scalar.add_instruction(mybir.InstLoadActFuncSet(
        name=nc.get_next_instruction_name(), act_func_set_id=6,
        ins=[], outs=[]))

    def chunk_steps(base, n):
        px = slice(base, base + n)

        xh = xin.tile([128, 2, n], F32R, tag="xh", name="xh")
        xv = x_d[:, px].rearrange("(h k) n -> k h n", h=2)
        nc.sync.dma_start(xh[:], xv)
        ps1 = pstat.tile([96, n], F32, tag="ps1", name="ps1", bufs=1)
        for kh in range(2):
            nc.tensor.matmul(
                out=ps1[:], lhsT=wred[:, kh, :],
                rhs=xh[:, kh, :],
                start=kh == 0, stop=kh == 1)
        yield

        # ---------------- con (f32r matmuls) + fp16 copy ----------------
        con = conp.tile([128, C, 4, n], F16, tag="con", name="con",
                        bufs=int(os.environ.get("EMBUFC", "4")))
        for c in range(C):
            for hp in range(2):
                pc = pcm.tile([128, 2, n], F32, tag="cm", name="pc", bufs=3)
                for gl in range(2):
                    gh = hp * 2 + gl
                    nc.tensor.matmul(
                        out=pc[:, gl, :], lhsT=wcon[:, gh, c, :],
                        rhs=xh[:, gh // 2, :],
                        start=True, stop=True)
                nc.scalar.copy(con[:, c, 2 * hp:2 * hp + 2], pc[:])
            yield

        # round-0 mean from presum (small) + broadcast
        s0 = small.tile([96, 3, n], F16, tag="s0", name="s0")
        t2 = small.tile([96, n], F32, tag="t2", name="t2")
        tv = small.tile([96, n], F32, tag="tv", name="tv")
        bc0 = bcp.tile([128, C, 3, n], F16, tag="bc0", name="bc0")
        with tc.high_priority(offset=HIPRI):
            nc.scalar.activation(s0[:, 0], ps1[:],
                                 mybir.ActivationFunctionType.Copy,
                                 scale=1.0 / G)
            for c in range(C):
                nc.sync.dma_start(
                    bc0[:, c, 0:1],
                    s0[c * 32:c * 32 + O, None, 0:1].to_broadcast(
                        (O, 8, 1, n)))
        yield

        mean0_bc = bc0[:, :, 0, None, :]

        # mtt kept across rounds is not needed; per-round tiles below
        sq_r = [None, None]
        m_r = [None, None]

        for rnd in range(2):
            last = rnd == 1
            if rnd == 0:
                bc = bc0
                # ------- d0 = con - mean0 ; sq0 = Square(d0) in-place ----
                d = big.tile([128, C, 4, n], F16, tag="dsq", name="d0",
                             bufs=int(os.environ.get("EMBUFS", "3")))
                nc.vector.tensor_tensor(
                    d[:, 2], con[:, 2],
                    bc[:, 2, 0, None, :].to_broadcast((128, 4, n)), sb)
                nc.vector.tensor_tensor(
                    d[:, 0:2], con[:, 0:2],
                    bc[:, 0:2, 0, None, :].to_broadcast((128, 2, 4, n)), sb)
                yield
                nc.scalar.activation(d[:, 2], d[:, 2], SQ)
                nc.scalar.activation(d[:, 0:2], d[:, 0:2], SQ)
                sq = d  # now holds sq0
                # S2'_0 = sum_g sq0
                ps2 = pstat.tile([96, n], F32, tag="ps2", name="ps2",
                                 bufs=1)
                for ci, c in enumerate((2, 0, 1)):
                    for gh in range(4):
                        nc.tensor.matmul(
                            out=ps2[:], lhsT=selred[:, c, :],
                            rhs=sq[:, c, gh, :],
                            start=(ci == 0 and gh == 0),
                            stop=(ci == 2 and gh == 3))
                yield
                # ------- var0 = S2'_0/G + EPS; hv, eu via Exp bias -------
                with tc.high_priority(offset=HIPRI):
                    nc.vector.tensor_scalar(
                        tv[:], ps2[:], 1.0 / G, EPS, op0=mm, op1=ad)
                    nc.scalar.activation(t2[:], tv[:], LN)
                    eu_k = EU_SCALE * (G / C + EPS)
                    nc.scalar.activation(s0[:, 2], t2[:], EXP, scale=-0.5,
                                         bias=lneukc[:])             # eu0
                    nc.scalar.activation(s0[:, 1], t2[:], EXP, scale=-1.0,
                                         bias=loghalfc[:])           # hv0
            else:
                bc = bcp.tile([128, C, 3, n], F16, tag="bc1", name="bc1")
                # ------- round-1 stats with centered-variance fixup ------
                rkr = small.tile([96, n], F32, tag="rkr", name="rkr")
                teu = small.tile([96, n], F32, tag="teu", name="teu")
                s1 = small.tile([96, 3, n], F16, tag="s1", name="s1")
                dlt = small.tile([96, n], F32, tag="dlt", name="dlt")
                with tc.high_priority(offset=HIPRI):
                    nc.scalar.activation(rkr[:], pr0[:, 0], mybir.ActivationFunctionType.Identity,
                                         bias=epsc[:])
                    nc.vector.reciprocal_approx_fast(rkr[:], rkr[:])
                    engK2 = (nc.gpsimd if os.environ.get("EMK2") == "1"
                             else nc.vector)
                    engK2.tensor_tensor(s1[:, 0], pr0[:, 1], rkr[:], mm)
                    nc.scalar.activation(teu[:], pr0[:, 0], mybir.ActivationFunctionType.Copy,
                                         scale=EU_SCALE)
                    nc.gpsimd.tensor_tensor(dlt[:], s1[:, 0], s0[:, 0], sb)
                    engK1 = (nc.gpsimd if os.environ.get("EMK1") == "1"
                             else nc.vector)
                    engK1.tensor_tensor(tv[:], psC[:], rkr[:], mm)
                    nc.gpsimd.tensor_tensor(dlt[:], dlt[:], dlt[:], mm)
                    nc.vector.scalar_tensor_tensor(
                        tv[:], tv[:], EPS, dlt[:], ad, sb)           # var1
                    nc.scalar.activation(t2[:], tv[:], LN)
                    nc.scalar.activation(s1[:, 1], t2[:], EXP, scale=-1.0,
                                         bias=loghalfc[:])           # hv1
                    nc.scalar.activation(tv[:], t2[:], EXP, scale=-0.5)
                    nc.gpsimd.tensor_tensor(s1[:, 2], teu[:], tv[:], mm)
                for c in range(C):
                    eng = nc.sync
                    eng.dma_start(
                        bc[:, c, 0:1],
                        s1[c * 32:c * 32 + O, None, 0:1].to_broadcast(
                            (O, 8, 1, n)))
                yield
                # d1 = con - mean1 (c01 on Pool via STT, c2 on DVE)
                d = big.tile([128, C, 4, n], F16, tag="dsq", name="d1",
                             bufs=int(os.environ.get("EMBUFS", "3")))
                nc.vector.tensor_tensor(
                    d[:, 2], con[:, 2],
                    bc[:, 2, 0, None, :].to_broadcast((128, 4, n)), sb)
                nc.gpsimd.tensor_tensor(
                    d[:, 0], con[:, 0],
                    bc[:, 0, 0, None, :].to_broadcast((128, 4, n)), sb)
                engA = nc.vector if os.environ.get("EMA", "1") == "1" else nc.gpsimd
                engA.tensor_tensor(
                    d[:, 1], con[:, 1],
                    bc[:, 1, 0, None, :].to_broadcast((128, 4, n)), sb)
                yield
                nc.scalar.activation(d[:, 2], d[:, 2], SQ)
                nc.scalar.activation(d[:, 0:2], d[:, 0:2], SQ)
                sq = d
                sq_r[1] = sq
                s0 = s1  # stats tile for broadcasts below

            # broadcast (hv, eu) per c
            for c in range(C):
                eng = nc.sync
                eng.dma_start(
                    bc[:, c, 1:3],
                    s0[c * 32:c * 32 + O, None, 1:3].to_broadcast(
                        (O, 8, 2, n)))
            yield

            # ---------------- E step ------------------------------------
            u = big.tile([128, C, 4, n], F16, tag="uEn",
                         name="u%d" % rnd,
                         bufs=int(os.environ.get("EMBUFU", "3")))
            nc.vector.tensor_tensor(
                u[:, 2], sq[:, 2],
                bc[:, 2, 1, None, :].to_broadcast((128, 4, n)), mm)
            nc.vector.tensor_tensor(
                u[:, 0:2], sq[:, 0:2],
                bc[:, 0:2, 1, None, :].to_broadcast((128, 2, 4, n)), mm)
            yield
            if os.environ.get("EMEC", "0") == "1":
                for c in range(C):
                    nc.scalar.activation(u[:, c], u[:, c], EXP, bias=shiftc,
                                         scale=-1.0)
            else:
                nc.scalar.activation(u[:, 0:2], u[:, 0:2], EXP, bias=shiftc,
                                     scale=-1.0)
                nc.scalar.activation(u[:, 2], u[:, 2], EXP, bias=shiftc,
                                     scale=-1.0)
            yield
            nc.vector.tensor_tensor(
                u[:, 0:2], u[:, 0:2],
                bc[:, 0:2, 2, None, :].to_broadcast((128, 2, 4, n)), mm)
            nc.vector.tensor_tensor(
                u[:, 2], u[:, 2],
                bc[:, 2, 2, None, :].to_broadcast((128, 4, n)), mm)
            yield
            # u now holds numer n
            n01 = big.tile([128, 4, n], F16, tag="n01", name="n01")
            den32 = big.tile([128, 4, n], F32, tag="den32", name="den32")
            with tc.high_priority(offset=HIPRI):
                if os.environ.get("EMNH", "0") == "1":
                    for h in range(2):
                        nc.vector.tensor_tensor(
                            n01[:, 2 * h:2 * h + 2], u[:, 0, 2 * h:2 * h + 2],
                            u[:, 1, 2 * h:2 * h + 2], ad)
                else:
                    nc.vector.tensor_tensor(n01[:], u[:, 0], u[:, 1], ad)
                if os.environ.get("EMDH", "1") == "1":
                    for h in range(2):
                        nc.gpsimd.tensor_tensor(
                            den32[:, 2 * h:2 * h + 2], n01[:, 2 * h:2 * h + 2],
                            u[:, 2, 2 * h:2 * h + 2], ad)
                        nc.vector.reciprocal_approx_fast(
                            den32[:, 2 * h:2 * h + 2],
                            den32[:, 2 * h:2 * h + 2])
                else:
                    nc.gpsimd.tensor_tensor(den32[:], n01[:], u[:, 2], ad)
                    nc.vector.reciprocal_approx_fast(den32[:], den32[:])
            yield
            # m = n*q (Pool c01, DVE c2), mc = m*con, mtt = m*sq
            rr = big.tile([128, C, 4, 2, n], F16, tag="rr", name="rr")
            if os.environ.get("EMMH", "0") == "1":
                for c in range(2):
                    for h in range(2):
                        nc.gpsimd.tensor_tensor(
                            rr[:, c, 2 * h:2 * h + 2, 0, :],
                            u[:, c, 2 * h:2 * h + 2],
                            den32[:, 2 * h:2 * h + 2, :], mm)
            else:
                for c in range(2):
                    nc.gpsimd.tensor_tensor(
                        rr[:, c, :, 0, :], u[:, c], den32[:, :, :], mm)
            nc.vector.tensor_tensor(
                rr[:, 2, :, 0, :], u[:, 2], den32[:], mm)
            yield
            nc.vector.tensor_tensor(
                rr[:, 2, :, 1, :], rr[:, 2, :, 0, :], con[:, 2], mm)
            nc.vector.tensor_tensor(
                rr[:, 0:2, :, 1, :], rr[:, 0:2, :, 0, :], con[:, 0:2], mm)
            if not last:
                mtt = big.tile([128, C, 4, n], F16, tag="mtt", name="mtt",
                               bufs=int(os.environ.get("EMBUFM", "2")))
                nc.vector.tensor_tensor(
                    mtt[:, 2], rr[:, 2, :, 0, :], sq[:, 2], mm)
                nc.gpsimd.tensor_tensor(
                    mtt[:, 0], rr[:, 0, :, 0, :], sq[:, 0], mm)
                engC = nc.vector if os.environ.get("EMC") == "1" else nc.gpsimd
                engC.tensor_tensor(
                    mtt[:, 1], rr[:, 1, :, 0, :], sq[:, 1], mm)
            yield

            # ---------------- reductions over g ---------------------------
            if not last:
                pr0 = pstat.tile([96, 2, n], F32, tag="pr0", name="pr0",
                                 bufs=1)
                red = pr0
            else:
                pr1 = pstat.tile([96, 2, n], F32, tag="pr1", name="pr1",
                                 bufs=1)
                red = pr1
            for ci, c in enumerate((2, 0, 1)):
                for gh in range(4):
                    nc.tensor.matmul(
                        out=red[:], lhsT=selred[:, c, :],
                        rhs=rr[:, c, gh, :, :],
                        start=(ci == 0 and gh == 0),
                        stop=(ci == 2 and gh == 3))
            if not last:
                psC = pstat.tile([96, n], F32, tag="psC", name="psC",
                                 bufs=1)
                for ci, c in enumerate((2, 0, 1)):
                    for gh in range(4):
                        nc.tensor.matmul(
                            out=psC[:], lhsT=selred[:, c, :],
                            rhs=mtt[:, c, gh, :],
                            start=(ci == 0 and gh == 0),
                            stop=(ci == 2 and gh == 3))
            yield

        # ---------------- final mean + bias -> out --------------------
        rk2 = small.tile([96, n], F32, tag="rk2", name="rk2")
        mf = small.tile([96, n], F32, tag="mf", name="mf")
        outsb = small.tile([96, n], F32, tag="outsb", name="outsb")
        nc.scalar.activation(rk2[:], pr1[:, 0], mybir.ActivationFunctionType.Identity, bias=epsc[:])
        nc.vector.reciprocal_approx_fast(rk2[:], rk2[:])
        nc.vector.tensor_tensor(mf[:], pr1[:, 1], rk2[:], mm)
        nc.scalar.activation(outsb[:], mf[:], mybir.ActivationFunctionType.Identity, bias=biasc[:])
        for c in range(C):
            nc.sync.dma_start(
                out_d[c * O:(c + 1) * O, px],
                outsb[c * 32:c * 32 + O, :])
        yield

    # software pipeline: interleave chunks' instruction streams
    WINDOW = WIN
    STAGGER = int(os.environ.get("EMSTAG", "6"))
    RAMP = os.environ.get("EMRAMP", "")
    ramp = ([int(v) for v in RAMP.split(",")] if RAMP else [])
    nadd = 0

    def cur_stagger():
        return ramp[nadd] if nadd < len(ramp) else STAGGER

    spans = []
    hm = os.environ.get("EMHALF", "0")
    if hm == "1":
        spans += [(0, N // 2), (N // 2, N // 2)]
        spans += [(c * N, N) for c in range(1, NCHUNK - 1)]
        spans += [((NCHUNK - 1) * N, N // 2),
                  ((NCHUNK - 1) * N + N // 2, N // 2)]
    elif hm == "2":
        spans += [(c * N, N) for c in range(NCHUNK - 1)]
        spans += [((NCHUNK - 1) * N, N // 2),
                  ((NCHUNK - 1) * N + N // 2, N // 2)]
    elif hm == "3":
        spans += [(0, N // 2), (N // 2, N // 2)]
        spans += [(c * N, N) for c in range(1, NCHUNK)]
    else:
        spans = [(c * N, N) for c in range(NCHUNK)]
    pending = [chunk_steps(b, n) for (b, n) in spans]
    active = []
    tick = 0
    last_add = -99
    while pending or active:
        if pending and len(active) < WINDOW and tick - last_add >= cur_stagger():
            active.append(pending.pop(0))
            last_add = tick
            nadd += 1
        if not active and pending:
            active.append(pending.pop(0))
            last_add = tick
            nadd += 1
        nxt = []
        for g in active:
            try:
                next(g)
                nxt.append(g)
            except StopIteration:
                pass
        active = nxt
        tick += 1

    ctx.close()


def _round_f32r(a):
    u = np.ascontiguousarray(a, dtype=np.float32).view(np.uint32)
    lsb = (u >> 12) & 1
    u = (u + 0x7FF + lsb) & np.uint32(0xFFFFF000)
    return u.view(np.float32)


def _prep_aux(weight, bias):
    wg = weight.reshape(C, O, G, I)
    wcon = np.zeros((128, 4, C, 128), np.float32)
    for gh in range(4):
        kh = gh // 2
        for c in range(C):
            for o in range(O):
                for gl in range(8):
                    g = gh * 8 + gl
                    g_rel = g - kh * 16
                    wcon[g_rel * 8:(g_rel + 1) * 8, gh, c, o * 8 + gl] = wg[c, o, g, :]
    # presummed weights: S1[c*32+o] = sum_g sum_i wg[c,o,g,i] x[g*8+i]
    wred = np.zeros((128, 2, 96), np.float32)
    for kh in range(2):
        for c in range(C):
            for o in range(O):
                for g_rel in range(16):
                    g = kh * 16 + g_rel
                    wred[g_rel * 8:(g_rel + 1) * 8, kh, c * 32 + o] = wg[c, o, g, :]
    selred = np.zeros((128, 3, 96), np.float16)
    for c in range(C):
        for o in range(O):
            selred[o * 8:(o + 1) * 8, c, c * 32 + o] = 1.0
    selred = selred.reshape(128, 3 * 96)
    biasc = np.zeros((128, 1), np.float32)
    for c in range(C):
        biasc[c * 32:c * 32 + O, 0] = bias[c * O:(c + 1) * O]
    shiftc = np.full((128, 1), SHIFT, np.float32)
    eu_k = EU_SCALE * (G / C + EPS)
    lneukc = np.full((128, 1), np.log(eu_k), np.float32)
    loghalfc = np.full((128, 1), LOG_HALF, np.float32)
    aux = _round_f32r(np.concatenate(
        [wcon.reshape(128, 4 * C * 128), wred.reshape(128, 2 * 96)], axis=1))
    epsc = np.full((128, 1), EPS, np.float32)
    auxf = np.concatenate([shiftc, biasc, lneukc, loghalfc, epsc], axis=1)
    return (np.ascontiguousarray(aux), np.ascontiguousarray(auxf),
            np.ascontiguousarray(selred))


_NC_CACHE = {}


def _get_nc():
    if "nc" not in _NC_CACHE:
        _NC_CACHE["nc"] = build_program()
    return _NC_CACHE["nc"]


def kernel(x, weight, bias, _trace=False, _trace_kwargs=None):
    x = np.ascontiguousarray(np.asarray(x, dtype=np.float32))
    weight = np.asarray(weight, dtype=np.float32)
    bias = np.asarray(bias, dtype=np.float32)

    aux, auxf, selred = _prep_aux(weight, bias)
    nc = _get_nc()

    xr = _round_f32r(x.reshape(B, GI, HW))
    in_maps = []
    for b in range(B):
        in_maps.append({
            "x": np.ascontiguousarray(xr[b]),
            "aux": aux, "auxf": auxf, "selred": selred,
        })
    res = run_bass_kernel_spmd(
        nc, in_maps, core_ids=list(range(B)),
        trace=_trace, **(_trace_kwargs or {}))

    out = np.stack([res.results[b]["out"].reshape(CO, H, W) for b in range(B)])
    if _trace:
        return out, res
    return out


if __name__ == "__main__":
    rng = np.random.default_rng(0)
    x = rng.standard_normal((B, GI, H, W), dtype=np.float32)
    w = rng.standard_normal((CO, GI), dtype=np.float32) * np.sqrt(2.0 / GI)
    bb = rng.standard_normal((CO,)).astype(np.float32) * 0.02
    out = kernel(x=x, weight=w, bias=bb)
    print("out", out.shape, out.dtype, np.abs(out).max())
